# revision 6
# baseline (speedup 1.0000x reference)
# Trainium2 Bass kernel for nn_CN_MLP_71631464563230 (moe_routing).
#
# Math (after folding the classifier into the mixture):
#   mlp_out = x @ W.T + b                      [B, H]
#   a       = mlp_out @ attn                   [B, T]
#   V[t,h]  = sum_k CM[t,h,k] * cla_w[k]       [T, H]   (computed on device)
#   s       = mlp_out @ V.T                    [B, T]
#   out[b]  = sigmoid( (sum_t e^{a_bt} s_bt) / (sum_t e^{a_bt}) + cla_b )
#
# Sharding: data-parallel over B across 8 cores (1024 rows/core); params
# replicated. Host-side prep is layout-only (transposes) so the contraction
# dims land on SBUF partitions.

import numpy as np

import concourse.bass as bass
import concourse.mybir as mybir
import concourse.tile as tile
from concourse import bacc
from concourse.bass_utils import run_bass_kernel_spmd
from concourse.masks import make_identity

B, D, H, T = 8192, 5000, 512, 16
NCORES = 8
BLOC = B // NCORES            # 1024 batch rows per core
NBSUB = BLOC // 128           # 8 b-subtiles per core
NHALF = 2                     # process b in two halves of 512
HALFW = BLOC // NHALF         # 512
KT = (D + 127) // 128         # 40 k-tiles over D
HT = H // 128                 # 4 h-tiles
KC = H // 128                 # 4 k-tiles over H (for V)
F32 = mybir.dt.float32
MM_DT = mybir.dt.float32r     # TF32-class matmul: 1 cyc/row at N>=256

LAST_RESULTS = None           # BassKernelResults from the most recent run


def _build_nc():
    nc = bacc.Bacc("TRN2", target_bir_lowering=False)

    xT = nc.dram_tensor("xT", [D, BLOC], MM_DT, kind="ExternalInput").ap()
    wT = nc.dram_tensor("wT", [D, H], MM_DT, kind="ExternalInput").ap()
    cmt = nc.dram_tensor("cmt", [H, T * H], MM_DT, kind="ExternalInput").ap()
    attn = nc.dram_tensor("attn", [H, T], MM_DT, kind="ExternalInput").ap()
    mlpb = nc.dram_tensor("mlp_b", [H], F32, kind="ExternalInput").ap()
    claM_d = nc.dram_tensor("claM", [128, KC, 2 * T], MM_DT, kind="ExternalInput").ap()
    clab = nc.dram_tensor("cla_b", [1], F32, kind="ExternalInput").ap()
    out = nc.dram_tensor("out", [128, NBSUB], F32, kind="ExternalOutput").ap()

    with tile.TileContext(nc) as tc:
        import contextlib

        ctx = contextlib.ExitStack()
        with ctx:
            singles = ctx.enter_context(tc.tile_pool(name="singles", bufs=1))
            xtp = ctx.enter_context(tc.tile_pool(name="xt", bufs=8))
            wtp = ctx.enter_context(tc.tile_pool(name="wt", bufs=1))
            cmtp = ctx.enter_context(tc.tile_pool(name="cmt", bufs=3))
            mop = ctx.enter_context(tc.tile_pool(name="mo", bufs=2))
            epp = ctx.enter_context(tc.tile_pool(name="ep", bufs=4))
            mmp = ctx.enter_context(tc.tile_pool(name="mm", bufs=1, space="PSUM"))
            vpsp = ctx.enter_context(tc.tile_pool(name="vps", bufs=1, space="PSUM"))
            tpp = ctx.enter_context(tc.tile_pool(name="tp", bufs=1, space="PSUM"))
            asp = ctx.enter_context(tc.tile_pool(name="as", bufs=2, space="PSUM"))

            # ---- small constants -------------------------------------------
            # attn+VT combined rhs per h-tile: cols 0:16 = attn, 16:32 = V.T
            as_rhs = []
            for h in range(HT):
                t_ = singles.tile([128, 2 * T], MM_DT, tag=f"asrhs{h}", name=f"asrhs{h}")
                nc.sync.dma_start(out=t_[:, 0:T], in_=attn[h * 128:(h + 1) * 128, :])
                as_rhs.append(t_)

            biasT = singles.tile([128, HT], F32, tag="biasT")
            nc.sync.dma_start(out=biasT, in_=mlpb.rearrange("(a p) -> p a", p=128))

            # cla_w chunks embedded (host-side) in a zero buffer so an offset
            # slice gives a [128, T] lhsT with cla in column t, zeros elsewhere.
            claM = singles.tile([128, KC, 2 * T], MM_DT, tag="claM")
            nc.sync.dma_start(out=claM, in_=claM_d)

            clab_sb = singles.tile([128, 1], F32, tag="clab")
            nc.sync.dma_start(
                out=clab_sb,
                in_=bass.AP(tensor=clab.tensor, offset=0, ap=[[0, 128], [1, 1]]),
            )

            ident = singles.tile([T, T], F32, tag="ident")
            make_identity(nc, ident)

            v_sb = singles.tile([T, H], F32, tag="v_sb")
            out_sb = singles.tile([128, NBSUB], F32, tag="out_sb")

            # resident mlp weights: one [128, 512] tile per k-tile
            wt_tiles = []
            for k in range(KT):
                dk = min(128, D - k * 128)
                t_ = wtp.tile([128, H], MM_DT, tag=f"wt{k}", name=f"wt{k}")
                nc.sync.dma_start(out=t_[:dk, :], in_=wT[k * 128:k * 128 + dk, :])
                wt_tiles.append(t_)

            # ---- main matmul: mlp_outT[h, b] = sum_d wT[d,h] * xT[d,b] ------
            mo = {}  # (half, h) -> sbuf [128, HALFW]
            mm_ps = {}
            for half in range(NHALF):
                for h in range(HT):
                    mm_ps[(half, h)] = mmp.tile([128, HALFW], F32, tag=f"mm{h}", name=f"mmps{half}_{h}")
                for k in range(KT):
                    dk = min(128, D - k * 128)
                    xt_t = xtp.tile([128, HALFW], MM_DT, tag="xt")
                    nc.sync.dma_start(
                        out=xt_t[:dk, :],
                        in_=xT[k * 128:k * 128 + dk,
                              half * HALFW:(half + 1) * HALFW],
                    )
                    for h in range(HT):
                        nc.tensor.matmul(
                            mm_ps[(half, h)],
                            lhsT=wt_tiles[k][:dk, h * 128:(h + 1) * 128],
                            rhs=xt_t[:dk, :],
                            start=(k == 0),
                            stop=(k == KT - 1),
                        )
                for h in range(HT):
                    m = mop.tile([128, HALFW], MM_DT, tag=f"mo{h}")
                    nc.vector.tensor_scalar_add(m, mm_ps[(half, h)], biasT[:, h:h + 1])
                    mo[(half, h)] = m

            # ---- V[t, h] = sum_k cla_w[k] * CM[t, h, k] ---------------------
            # cmt is CM rearranged to [k, t*H + h]; accumulate rows of V into
            # one [T, H] psum: lhsT column t holds cla chunk j, rest zeros.
            v_ps = vpsp.tile([T, H], F32, tag="v_ps")
            for t in range(T):
                cmt_t = cmtp.tile([128, KC, H], MM_DT, tag="cmt")
                nc.sync.dma_start(
                    out=cmt_t,
                    in_=cmt[:, t * H:(t + 1) * H].rearrange("(j p) h -> p j h", p=128),
                )
                for j in range(KC):
                    nc.tensor.matmul(
                        v_ps,
                        lhsT=claM[:, j, T - t:2 * T - t],
                        rhs=cmt_t[:, j, :],
                        start=(t == 0 and j == 0),
                        stop=(t == T - 1 and j == KC - 1),
                    )
            nc.vector.tensor_copy(v_sb, v_ps)
            for h in range(HT):
                tp_ps = tpp.tile([128, T], F32, tag="tp")
                nc.tensor.transpose(tp_ps, v_sb[:, h * 128:(h + 1) * 128], ident)
                nc.vector.tensor_copy(as_rhs[h][:, T:2 * T], tp_ps)

            # ---- epilogue: a | s, softmax-combine, sigmoid ------------------
            for half in range(NHALF):
                for jl in range(NBSUB // NHALF):
                    g = half * (NBSUB // NHALF) + jl
                    as_ps = asp.tile([128, 2 * T], F32, tag="as")
                    for h in range(HT):
                        nc.tensor.matmul(
                            as_ps,
                            lhsT=mo[(half, h)][:, jl * 128:(jl + 1) * 128],
                            rhs=as_rhs[h],
                            start=(h == 0),
                            stop=(h == HT - 1),
                        )
                    E = epp.tile([128, T], F32, tag="E")
                    den = epp.tile([128, 1], F32, tag="den")
                    nc.scalar.activation(
                        E, as_ps[:, 0:T], mybir.ActivationFunctionType.Exp,
                        accum_out=den,
                    )
                    prod = epp.tile([128, T], F32, tag="prod")
                    num = epp.tile([128, 1], F32, tag="num")
                    nc.vector.tensor_mul(prod, E, as_ps[:, T:2 * T])
                    nc.vector.reduce_sum(num, prod, axis=mybir.AxisListType.X)
                    rden = epp.tile([128, 1], F32, tag="rden")
                    nc.vector.reciprocal(rden, den)
                    lg = epp.tile([128, 1], F32, tag="lg")
                    nc.vector.tensor_mul(lg, num, rden)
                    nc.scalar.activation(
                        out_sb[:, g:g + 1], lg,
                        mybir.ActivationFunctionType.Sigmoid,
                        bias=clab_sb, scale=1.0,
                    )

            nc.sync.dma_start(out=out, in_=out_sb)

    nc.finalize()
    return nc


_NC_CACHE = None


def kernel(data_input, mlp_w, mlp_b, CM, attn, cla_w, cla_b):
    global LAST_RESULTS, _NC_CACHE

    data_input = np.ascontiguousarray(np.asarray(data_input, dtype=np.float32))
    mlp_w = np.asarray(mlp_w, dtype=np.float32)
    mlp_b = np.ascontiguousarray(np.asarray(mlp_b, dtype=np.float32))
    CM = np.asarray(CM, dtype=np.float32)
    attn = np.ascontiguousarray(np.asarray(attn, dtype=np.float32))
    cla_w = np.ascontiguousarray(np.asarray(cla_w, dtype=np.float32).reshape(H))
    cla_b = np.ascontiguousarray(np.asarray(cla_b, dtype=np.float32).reshape(1))

    wT = np.ascontiguousarray(mlp_w.T)                       # [D, H]
    cmt = np.ascontiguousarray(CM.transpose(2, 0, 1).reshape(H, T * H))
    claM = np.zeros((128, KC, 2 * T), dtype=np.float32)
    claM[:, :, T] = cla_w.reshape(KC, 128).T

    in_maps = []
    for i in range(NCORES):
        xT_i = np.ascontiguousarray(data_input[i * BLOC:(i + 1) * BLOC].T)
        in_maps.append({
            "xT": xT_i, "wT": wT, "cmt": cmt, "attn": attn,
            "mlp_b": mlp_b, "claM": claM, "cla_b": cla_b,
        })

    if _NC_CACHE is None:
        _NC_CACHE = _build_nc()

    import os
    trace = bool(int(os.environ.get("KERNEL_TRACE", "0")))
    res = run_bass_kernel_spmd(
        _NC_CACHE, in_maps, core_ids=list(range(NCORES)), trace=trace,
        trace_cores=[0] if trace else None,
    )
    LAST_RESULTS = res

    full = np.empty(B, dtype=np.float32)
    for i in range(NCORES):
        full[i * BLOC:(i + 1) * BLOC] = res.results[i]["out"].T.reshape(BLOC)
    return full


# revision 8
# speedup vs baseline: 1.3706x; 1.3706x over previous
# Trainium2 Bass kernel for nn_CN_MLP_71631464563230 (moe_routing).
#
# Math (after folding the classifier into the mixture):
#   mlp_out = x @ W.T + b                      [B, H]
#   a       = mlp_out @ attn                   [B, T]
#   V[t,h]  = sum_k CM[t,h,k] * cla_w[k]       [T, H]   (computed on device)
#   s       = mlp_out @ V.T                    [B, T]
#   out[b]  = sigmoid( (sum_t e^{a_bt} s_bt) / (sum_t e^{a_bt}) + cla_b )
#
# Sharding: data-parallel over B across 8 cores (1024 rows/core); params
# replicated. Host-side prep is layout-only (transposes + optional dtype
# narrowing) so contraction dims land on SBUF partitions.

import os

import ml_dtypes
import numpy as np

import concourse.bass as bass
import concourse.mybir as mybir
import concourse.tile as tile
from concourse import bacc
from concourse.bass_utils import run_bass_kernel_spmd
from concourse.masks import make_identity

B, D, H, T = 8192, 5000, 512, 16
NCORES = 8
BLOC = B // NCORES            # 1024 batch rows per core
NBSUB = BLOC // 128           # 8 b-subtiles per core
NHALF = 2                     # process b in two halves of 512
HALFW = BLOC // NHALF         # 512
KT = (D + 127) // 128         # 40 k-tiles over D
HT = H // 128                 # 4 h-tiles
KC = H // 128                 # 4 k-tiles over H (for V)
F32 = mybir.dt.float32

USE_BF16 = os.environ.get("KERNEL_BF16", "1") == "1"
MM_DT = mybir.dt.bfloat16 if USE_BF16 else mybir.dt.float32r
NP_MM = ml_dtypes.bfloat16 if USE_BF16 else np.float32

LAST_RESULTS = None           # BassKernelResults from the most recent run


def _build_nc():
    nc = bacc.Bacc("TRN2", target_bir_lowering=False)

    xT = nc.dram_tensor("xT", [D, BLOC], MM_DT, kind="ExternalInput").ap()
    wT = nc.dram_tensor("wT", [D, H], MM_DT, kind="ExternalInput").ap()
    cmt = nc.dram_tensor("cmt", [H, T * H], MM_DT, kind="ExternalInput").ap()
    attn = nc.dram_tensor("attn", [H, T], MM_DT, kind="ExternalInput").ap()
    mlpb = nc.dram_tensor("mlp_b", [H], F32, kind="ExternalInput").ap()
    claM_d = nc.dram_tensor("claM", [128, KC, 2 * T], MM_DT, kind="ExternalInput").ap()
    clab = nc.dram_tensor("cla_b", [1], F32, kind="ExternalInput").ap()
    out = nc.dram_tensor("out", [128, NBSUB], F32, kind="ExternalOutput").ap()

    with tile.TileContext(nc) as tc:
        import contextlib

        ctx = contextlib.ExitStack()
        with ctx:
            singles = ctx.enter_context(tc.tile_pool(name="singles", bufs=1))
            xtp = ctx.enter_context(tc.tile_pool(name="xt", bufs=8))
            wtp = ctx.enter_context(tc.tile_pool(name="wt", bufs=1))
            cmtp = ctx.enter_context(tc.tile_pool(name="cmt", bufs=3))
            mop = ctx.enter_context(tc.tile_pool(name="mo", bufs=2))
            epp = ctx.enter_context(tc.tile_pool(name="ep", bufs=4))
            mmp = ctx.enter_context(tc.tile_pool(name="mm", bufs=1, space="PSUM"))
            vpsp = ctx.enter_context(tc.tile_pool(name="vps", bufs=1, space="PSUM"))
            tpp = ctx.enter_context(tc.tile_pool(name="tp", bufs=1, space="PSUM"))
            asp = ctx.enter_context(tc.tile_pool(name="as", bufs=2, space="PSUM"))

            # ---- small constants -------------------------------------------
            # attn+VT combined rhs per h-tile: cols 0:16 = attn, 16:32 = V.T
            as_rhs = []
            for h in range(HT):
                t_ = singles.tile([128, 2 * T], MM_DT, tag=f"asrhs{h}", name=f"asrhs{h}")
                nc.sync.dma_start(out=t_[:, 0:T], in_=attn[h * 128:(h + 1) * 128, :])
                as_rhs.append(t_)

            biasT = singles.tile([128, HT], F32, tag="biasT")
            nc.sync.dma_start(out=biasT, in_=mlpb.rearrange("(a p) -> p a", p=128))

            # cla_w chunks embedded (host-side) in a zero buffer so an offset
            # slice gives a [128, T] lhsT with cla in column t, zeros elsewhere.
            claM = singles.tile([128, KC, 2 * T], MM_DT, tag="claM")
            nc.sync.dma_start(out=claM, in_=claM_d)

            clab_sb = singles.tile([128, 1], F32, tag="clab")
            nc.gpsimd.dma_start(
                out=clab_sb,
                in_=bass.AP(tensor=clab.tensor, offset=0, ap=[[0, 128], [1, 1]]),
            )

            ident = singles.tile([T, T], MM_DT, tag="ident")
            make_identity(nc, ident)

            v_sb = singles.tile([T, H], MM_DT, tag="v_sb")
            out_sb = singles.tile([128, NBSUB], F32, tag="out_sb")

            v_ps = vpsp.tile([T, H], F32, tag="v_ps")
            wt_tiles = [None] * KT
            mo = {}
            mm_ps = {}

            # V work interleaved into the main loop: one t-group every 5th
            # (half, k) slot keeps cmt DMA spread out and the PE warm.
            def v_group(t):
                cmt_t = cmtp.tile([128, KC, H], MM_DT, tag="cmt", name=f"cmt{t}")
                nc.sync.dma_start(
                    out=cmt_t,
                    in_=cmt[:, t * H:(t + 1) * H].rearrange("(j p) h -> p j h", p=128),
                )
                for j in range(KC):
                    nc.tensor.matmul(
                        v_ps,
                        lhsT=claM[:, j, T - t:2 * T - t],
                        rhs=cmt_t[:, j, :],
                        start=(t == 0 and j == 0),
                        stop=(t == T - 1 and j == KC - 1),
                    )

            # ---- main matmul: mlp_outT[h, b] = sum_d wT[d,h] * xT[d,b] ------
            for half in range(NHALF):
                for h in range(HT):
                    mm_ps[(half, h)] = mmp.tile(
                        [128, HALFW], F32, tag=f"mm{h}", name=f"mmps{half}_{h}"
                    )
                for k in range(KT):
                    slot = half * KT + k
                    if slot % 5 == 0 and slot // 5 < T:
                        v_group(slot // 5)
                    dk = min(128, D - k * 128)
                    if half == 0:
                        wt_tiles[k] = wtp.tile([128, H], MM_DT, tag=f"wt{k}", name=f"wt{k}")
                        nc.sync.dma_start(
                            out=wt_tiles[k][:dk, :], in_=wT[k * 128:k * 128 + dk, :]
                        )
                    xt_t = xtp.tile([128, HALFW], MM_DT, tag="xt")
                    nc.sync.dma_start(
                        out=xt_t[:dk, :],
                        in_=xT[k * 128:k * 128 + dk,
                              half * HALFW:(half + 1) * HALFW],
                    )
                    for h in range(HT):
                        nc.tensor.matmul(
                            mm_ps[(half, h)],
                            lhsT=wt_tiles[k][:dk, h * 128:(h + 1) * 128],
                            rhs=xt_t[:dk, :],
                            start=(k == 0),
                            stop=(k == KT - 1),
                        )
                for h in range(HT):
                    m = mop.tile([128, HALFW], MM_DT, tag=f"mo{h}", name=f"mo{half}_{h}")
                    nc.vector.tensor_scalar_add(m, mm_ps[(half, h)], biasT[:, h:h + 1])
                    mo[(half, h)] = m

            # ---- finish V: copy to SBUF, transpose into as_rhs cols 16:32 ---
            nc.vector.tensor_copy(v_sb, v_ps)
            for h in range(HT):
                tp_ps = tpp.tile([128, T], MM_DT, tag="tp")
                nc.tensor.transpose(tp_ps, v_sb[:, h * 128:(h + 1) * 128], ident)
                nc.vector.tensor_copy(as_rhs[h][:, T:2 * T], tp_ps)

            # ---- epilogue: a | s, softmax-combine, sigmoid ------------------
            for half in range(NHALF):
                for jl in range(NBSUB // NHALF):
                    g = half * (NBSUB // NHALF) + jl
                    as_ps = asp.tile([128, 2 * T], F32, tag="as")
                    for h in range(HT):
                        nc.tensor.matmul(
                            as_ps,
                            lhsT=mo[(half, h)][:, jl * 128:(jl + 1) * 128],
                            rhs=as_rhs[h],
                            start=(h == 0),
                            stop=(h == HT - 1),
                        )
                    E = epp.tile([128, T], F32, tag="E")
                    den = epp.tile([128, 1], F32, tag="den")
                    nc.scalar.activation(
                        E, as_ps[:, 0:T], mybir.ActivationFunctionType.Exp,
                        accum_out=den,
                    )
                    prod = epp.tile([128, T], F32, tag="prod")
                    num = epp.tile([128, 1], F32, tag="num")
                    nc.vector.tensor_mul(prod, E, as_ps[:, T:2 * T])
                    nc.vector.reduce_sum(num, prod, axis=mybir.AxisListType.X)
                    rden = epp.tile([128, 1], F32, tag="rden")
                    nc.vector.reciprocal(rden, den)
                    lg = epp.tile([128, 1], F32, tag="lg")
                    nc.vector.tensor_mul(lg, num, rden)
                    nc.scalar.activation(
                        out_sb[:, g:g + 1], lg,
                        mybir.ActivationFunctionType.Sigmoid,
                        bias=clab_sb, scale=1.0,
                    )

            nc.sync.dma_start(out=out, in_=out_sb)

    nc.finalize()
    return nc


_NC_CACHE = None


def kernel(data_input, mlp_w, mlp_b, CM, attn, cla_w, cla_b):
    global LAST_RESULTS, _NC_CACHE

    data_input = np.ascontiguousarray(np.asarray(data_input, dtype=np.float32))
    mlp_w = np.asarray(mlp_w, dtype=np.float32)
    mlp_b = np.ascontiguousarray(np.asarray(mlp_b, dtype=np.float32))
    CM = np.asarray(CM, dtype=np.float32)
    attn_np = np.ascontiguousarray(np.asarray(attn, dtype=np.float32)).astype(NP_MM)
    cla_w = np.ascontiguousarray(np.asarray(cla_w, dtype=np.float32).reshape(H))
    cla_b = np.ascontiguousarray(np.asarray(cla_b, dtype=np.float32).reshape(1))

    wT = np.ascontiguousarray(mlp_w.T).astype(NP_MM)             # [D, H]
    cmt = np.ascontiguousarray(
        CM.transpose(2, 0, 1).reshape(H, T * H)
    ).astype(NP_MM)
    claM = np.zeros((128, KC, 2 * T), dtype=np.float32)
    claM[:, :, T] = cla_w.reshape(KC, 128).T
    claM = claM.astype(NP_MM)

    in_maps = []
    for i in range(NCORES):
        xT_i = np.ascontiguousarray(data_input[i * BLOC:(i + 1) * BLOC].T).astype(NP_MM)
        in_maps.append({
            "xT": xT_i, "wT": wT, "cmt": cmt, "attn": attn_np,
            "mlp_b": mlp_b, "claM": claM, "cla_b": cla_b,
        })

    if _NC_CACHE is None:
        _NC_CACHE = _build_nc()

    trace = bool(int(os.environ.get("KERNEL_TRACE", "0")))
    res = run_bass_kernel_spmd(
        _NC_CACHE, in_maps, core_ids=list(range(NCORES)), trace=trace,
        trace_cores=[0] if trace else None,
    )
    LAST_RESULTS = res

    full = np.empty(B, dtype=np.float32)
    for i in range(NCORES):
        full[i * BLOC:(i + 1) * BLOC] = res.results[i]["out"].T.reshape(BLOC)
    return full


# revision 10
# speedup vs baseline: 1.4096x; 1.0284x over previous
# Trainium2 Bass kernel for nn_CN_MLP_71631464563230 (moe_routing).
#
# Math (after folding the classifier into the mixture):
#   mlp_out = x @ W.T + b                      [B, H]
#   a       = mlp_out @ attn                   [B, T]
#   V[t,h]  = sum_k CM[t,h,k] * cla_w[k]       [T, H]   (computed on device)
#   s       = mlp_out @ V.T                    [B, T]
#   out[b]  = sigmoid( (sum_t e^{a_bt} s_bt) / (sum_t e^{a_bt}) + cla_b )
#
# Sharding: data-parallel over B across 8 cores (1024 rows/core); params
# replicated. Host-side prep is layout-only (transposes + optional dtype
# narrowing) so contraction dims land on SBUF partitions.

import os

import ml_dtypes
import numpy as np

import concourse.bass as bass
import concourse.mybir as mybir
import concourse.tile as tile
from concourse import bacc
from concourse.bass_utils import run_bass_kernel_spmd
from concourse.masks import make_identity

B, D, H, T = 8192, 5000, 512, 16
NCORES = 8
BLOC = B // NCORES            # 1024 batch rows per core
NBSUB = BLOC // 128           # 8 b-subtiles per core
NHALF = 2                     # process b in two halves of 512
HALFW = BLOC // NHALF         # 512
KT = (D + 127) // 128         # 40 k-tiles over D
HT = H // 128                 # 4 h-tiles
KC = H // 128                 # 4 k-tiles over H (for V)
F32 = mybir.dt.float32

USE_BF16 = os.environ.get("KERNEL_BF16", "1") == "1"
MM_DT = mybir.dt.bfloat16 if USE_BF16 else mybir.dt.float32r
NP_MM = ml_dtypes.bfloat16 if USE_BF16 else np.float32

LAST_RESULTS = None           # BassKernelResults from the most recent run


def _build_nc_f32r():
    nc = bacc.Bacc("TRN2", target_bir_lowering=False)

    xT = nc.dram_tensor("xT", [D, BLOC], MM_DT, kind="ExternalInput").ap()
    wT = nc.dram_tensor("wT", [D, H], MM_DT, kind="ExternalInput").ap()
    cmt = nc.dram_tensor("cmt", [H, T * H], MM_DT, kind="ExternalInput").ap()
    attn = nc.dram_tensor("attn", [H, T], MM_DT, kind="ExternalInput").ap()
    mlpb = nc.dram_tensor("mlp_b", [H], F32, kind="ExternalInput").ap()
    claM_d = nc.dram_tensor("claM", [128, KC, 2 * T], MM_DT, kind="ExternalInput").ap()
    clab = nc.dram_tensor("cla_b", [1], F32, kind="ExternalInput").ap()
    out = nc.dram_tensor("out", [128, NBSUB], F32, kind="ExternalOutput").ap()

    with tile.TileContext(nc) as tc:
        import contextlib

        ctx = contextlib.ExitStack()
        with ctx:
            singles = ctx.enter_context(tc.tile_pool(name="singles", bufs=1))
            xtp = ctx.enter_context(tc.tile_pool(name="xt", bufs=8))
            wtp = ctx.enter_context(tc.tile_pool(name="wt", bufs=1))
            cmtp = ctx.enter_context(tc.tile_pool(name="cmt", bufs=3))
            mop = ctx.enter_context(tc.tile_pool(name="mo", bufs=2))
            epp = ctx.enter_context(tc.tile_pool(name="ep", bufs=4))
            mmp = ctx.enter_context(tc.tile_pool(name="mm", bufs=2, space="PSUM"))
            vpsp = ctx.enter_context(tc.tile_pool(name="vps", bufs=1, space="PSUM"))
            tpp = ctx.enter_context(tc.tile_pool(name="tp", bufs=1, space="PSUM"))
            asp = ctx.enter_context(tc.tile_pool(name="as", bufs=2, space="PSUM"))

            # ---- small constants -------------------------------------------
            # attn+VT combined rhs per h-tile: cols 0:16 = attn, 16:32 = V.T
            as_rhs = []
            for h in range(HT):
                t_ = singles.tile([128, 2 * T], MM_DT, tag=f"asrhs{h}", name=f"asrhs{h}")
                nc.sync.dma_start(out=t_[:, 0:T], in_=attn[h * 128:(h + 1) * 128, :])
                as_rhs.append(t_)

            biasT = singles.tile([128, HT], F32, tag="biasT")
            nc.sync.dma_start(out=biasT, in_=mlpb.rearrange("(a p) -> p a", p=128))

            # cla_w chunks embedded (host-side) in a zero buffer so an offset
            # slice gives a [128, T] lhsT with cla in column t, zeros elsewhere.
            claM = singles.tile([128, KC, 2 * T], MM_DT, tag="claM")
            nc.sync.dma_start(out=claM, in_=claM_d)

            clab_sb = singles.tile([128, 1], F32, tag="clab")
            nc.gpsimd.dma_start(
                out=clab_sb,
                in_=bass.AP(tensor=clab.tensor, offset=0, ap=[[0, 128], [1, 1]]),
            )

            ident = singles.tile([T, T], MM_DT, tag="ident")
            make_identity(nc, ident)

            v_sb = singles.tile([T, H], MM_DT, tag="v_sb")
            out_sb = singles.tile([128, NBSUB], F32, tag="out_sb")

            v_ps = vpsp.tile([T, H], F32, tag="v_ps")
            wt_tiles = [None] * KT
            mo = {}
            mm_ps = {}

            # V work interleaved into the main loop: one t-group every 5th
            # (half, k) slot keeps cmt DMA spread out and the PE warm.
            def v_group(t):
                cmt_t = cmtp.tile([128, KC, H], MM_DT, tag="cmt", name=f"cmt{t}")
                nc.sync.dma_start(
                    out=cmt_t,
                    in_=cmt[:, t * H:(t + 1) * H].rearrange("(j p) h -> p j h", p=128),
                )
                for j in range(KC):
                    nc.tensor.matmul(
                        v_ps,
                        lhsT=claM[:, j, T - t:2 * T - t],
                        rhs=cmt_t[:, j, :],
                        start=(t == 0 and j == 0),
                        stop=(t == T - 1 and j == KC - 1),
                    )

            # ---- main matmul: mlp_outT[h, b] = sum_d wT[d,h] * xT[d,b] ------
            for half in range(NHALF):
                for h in range(HT):
                    mm_ps[(half, h)] = mmp.tile(
                        [128, HALFW], F32, tag=f"mm{h}", name=f"mmps{half}_{h}"
                    )
                for k in range(KT):
                    slot = half * KT + k
                    if slot % 5 == 0 and slot // 5 < T:
                        v_group(slot // 5)
                    dk = min(128, D - k * 128)
                    if half == 0:
                        wt_tiles[k] = wtp.tile([128, H], MM_DT, tag=f"wt{k}", name=f"wt{k}")
                        nc.sync.dma_start(
                            out=wt_tiles[k][:dk, :], in_=wT[k * 128:k * 128 + dk, :]
                        )
                    xt_t = xtp.tile([128, HALFW], MM_DT, tag="xt")
                    nc.sync.dma_start(
                        out=xt_t[:dk, :],
                        in_=xT[k * 128:k * 128 + dk,
                              half * HALFW:(half + 1) * HALFW],
                    )
                    for h in range(HT):
                        nc.tensor.matmul(
                            mm_ps[(half, h)],
                            lhsT=wt_tiles[k][:dk, h * 128:(h + 1) * 128],
                            rhs=xt_t[:dk, :],
                            start=(k == 0),
                            stop=(k == KT - 1),
                        )
                for h in range(HT):
                    m = mop.tile([128, HALFW], MM_DT, tag=f"mo{h}", name=f"mo{half}_{h}")
                    nc.vector.tensor_scalar_add(m, mm_ps[(half, h)], biasT[:, h:h + 1])
                    mo[(half, h)] = m

            # ---- finish V: copy to SBUF, transpose into as_rhs cols 16:32 ---
            nc.vector.tensor_copy(v_sb, v_ps)
            for h in range(HT):
                tp_ps = tpp.tile([128, T], MM_DT, tag="tp")
                nc.tensor.transpose(tp_ps, v_sb[:, h * 128:(h + 1) * 128], ident)
                nc.vector.tensor_copy(as_rhs[h][:, T:2 * T], tp_ps)

            # ---- epilogue: a | s, softmax-combine, sigmoid ------------------
            for half in range(NHALF):
                for jl in range(NBSUB // NHALF):
                    g = half * (NBSUB // NHALF) + jl
                    as_ps = asp.tile([128, 2 * T], F32, tag="as")
                    for h in range(HT):
                        nc.tensor.matmul(
                            as_ps,
                            lhsT=mo[(half, h)][:, jl * 128:(jl + 1) * 128],
                            rhs=as_rhs[h],
                            start=(h == 0),
                            stop=(h == HT - 1),
                        )
                    E = epp.tile([128, T], F32, tag="E")
                    den = epp.tile([128, 1], F32, tag="den")
                    nc.scalar.activation(
                        E, as_ps[:, 0:T], mybir.ActivationFunctionType.Exp,
                        accum_out=den,
                    )
                    prod = epp.tile([128, T], F32, tag="prod")
                    num = epp.tile([128, 1], F32, tag="num")
                    nc.vector.tensor_mul(prod, E, as_ps[:, T:2 * T])
                    nc.vector.reduce_sum(num, prod, axis=mybir.AxisListType.X)
                    rden = epp.tile([128, 1], F32, tag="rden")
                    nc.vector.reciprocal(rden, den)
                    lg = epp.tile([128, 1], F32, tag="lg")
                    nc.vector.tensor_mul(lg, num, rden)
                    nc.scalar.activation(
                        out_sb[:, g:g + 1], lg,
                        mybir.ActivationFunctionType.Sigmoid,
                        bias=clab_sb, scale=1.0,
                    )

            nc.sync.dma_start(out=out, in_=out_sb)

    nc.finalize()
    return nc


def _build_nc_bf16():
    BF = mybir.dt.bfloat16
    nc = bacc.Bacc("TRN2", target_bir_lowering=False)

    xT = nc.dram_tensor("xT", [D, BLOC], BF, kind="ExternalInput").ap()
    wT = nc.dram_tensor("wT", [D, H], BF, kind="ExternalInput").ap()
    cmn = nc.dram_tensor("cmn", [T, H, H], BF, kind="ExternalInput").ap()
    attn = nc.dram_tensor("attn", [H, T], BF, kind="ExternalInput").ap()
    mlpb = nc.dram_tensor("mlp_b", [H], F32, kind="ExternalInput").ap()
    claw = nc.dram_tensor("cla_w", [H], BF, kind="ExternalInput").ap()
    clab = nc.dram_tensor("cla_b", [1], F32, kind="ExternalInput").ap()
    out = nc.dram_tensor("out", [128, NBSUB], F32, kind="ExternalOutput").ap()

    with tile.TileContext(nc) as tc:
        import contextlib

        ctx = contextlib.ExitStack()
        with ctx:
            singles = ctx.enter_context(tc.tile_pool(name="singles", bufs=1))
            xtp = ctx.enter_context(tc.tile_pool(name="xt", bufs=10))
            wtp = ctx.enter_context(tc.tile_pool(name="wt", bufs=1))
            cmp_ = ctx.enter_context(tc.tile_pool(name="cm", bufs=3))
            vprodp = ctx.enter_context(tc.tile_pool(name="vprod", bufs=3))
            mop = ctx.enter_context(tc.tile_pool(name="mo", bufs=2))
            epp = ctx.enter_context(tc.tile_pool(name="ep", bufs=4))
            mmp = ctx.enter_context(tc.tile_pool(name="mm", bufs=2, space="PSUM"))

            # ---- small constants -------------------------------------------
            as_rhs = []
            for h in range(HT):
                t_ = singles.tile([128, 2 * T], BF, tag=f"asrhs{h}", name=f"asrhs{h}")
                nc.sync.dma_start(out=t_[:, 0:T], in_=attn[h * 128:(h + 1) * 128, :])
                as_rhs.append(t_)

            biasT = singles.tile([128, HT], F32, tag="biasT")
            nc.sync.dma_start(out=biasT, in_=mlpb.rearrange("(a p) -> p a", p=128))

            clab_sb = singles.tile([128, 1], F32, tag="clab")
            nc.gpsimd.dma_start(
                out=clab_sb,
                in_=bass.AP(tensor=clab.tensor, offset=0, ap=[[0, 128], [1, 1]]),
            )

            # cla_w replicated across partitions for the V elementwise product
            cla_rep = singles.tile([128, H], BF, tag="cla_rep")
            nc.sync.dma_start(out=cla_rep[0:1, :], in_=claw[None, :])
            nc.gpsimd.partition_broadcast(cla_rep, cla_rep[0:1, :])

            # V.T accumulates here (f32), cast into as_rhs cols 16:32 at the end
            vt_f32 = []
            for h in range(HT):
                v_ = singles.tile([128, T], F32, tag=f"vt{h}", name=f"vt{h}")
                vt_f32.append(v_)

            out_sb = singles.tile([128, NBSUB], F32, tag="out_sb")

            # V unit on DVE: VT[h, t] = sum_k CM[t, h, k] * cla_w[k]
            def v_unit(t):
                cm_t = cmp_.tile([128, HT, H], BF, tag="cm", name=f"cm{t}")
                nc.sync.dma_start(
                    out=cm_t, in_=cmn[t].rearrange("(j p) k -> p j k", p=128)
                )
                for j in range(HT):
                    prod = vprodp.tile([128, H], F32, tag="vprod", name=f"vp{t}_{j}")
                    nc.vector.tensor_mul(prod, cm_t[:, j, :], cla_rep)
                    nc.vector.reduce_sum(
                        vt_f32[j][:, t:t + 1], prod, axis=mybir.AxisListType.X
                    )

            # ---- main matmul in two b-halves: mlp_outT[h, b] ---------------
            wt_tiles = [None] * KT
            mm_ps = {}
            mo = {}
            for half in range(NHALF):
                for h in range(HT):
                    mm_ps[(half, h)] = mmp.tile(
                        [128, HALFW], F32, tag=f"mm{h}", name=f"mmps{half}_{h}"
                    )
                for k in range(KT):
                    slot = half * KT + k
                    if slot % 5 == 0 and slot // 5 < T:
                        v_unit(slot // 5)
                    dk = min(128, D - k * 128)
                    if half == 0:
                        wt_tiles[k] = wtp.tile([128, H], BF, tag=f"wt{k}", name=f"wt{k}")
                        nc.sync.dma_start(
                            out=wt_tiles[k][:dk, :], in_=wT[k * 128:k * 128 + dk, :]
                        )
                    xt_t = xtp.tile([128, HALFW], BF, tag="xt")
                    nc.sync.dma_start(
                        out=xt_t[:dk, :],
                        in_=xT[k * 128:k * 128 + dk,
                              half * HALFW:(half + 1) * HALFW],
                    )
                    for h in range(HT):
                        nc.tensor.matmul(
                            mm_ps[(half, h)],
                            lhsT=wt_tiles[k][:dk, h * 128:(h + 1) * 128],
                            rhs=xt_t[:dk, :],
                            start=(k == 0),
                            stop=(k == KT - 1),
                        )
                for h in range(HT):
                    m = mop.tile([128, HALFW], BF, tag=f"mo{h}", name=f"mo{half}_{h}")
                    nc.vector.tensor_scalar_add(m, mm_ps[(half, h)], biasT[:, h:h + 1])
                    mo[(half, h)] = m
            for h in range(HT):
                nc.vector.tensor_copy(as_rhs[h][:, T:2 * T], vt_f32[h])

            # ---- epilogue: a | s, softmax-combine, sigmoid ------------------
            for g in range(NBSUB):
                half, jl = divmod(g, NBSUB // NHALF)
                as_ps = mmp.tile([128, 2 * T], F32, tag=f"mm{g % HT}", name=f"as{g}")
                for h in range(HT):
                    nc.tensor.matmul(
                        as_ps,
                        lhsT=mo[(half, h)][:, jl * 128:(jl + 1) * 128],
                        rhs=as_rhs[h],
                        start=(h == 0),
                        stop=(h == HT - 1),
                    )
                E = epp.tile([128, T], F32, tag="E")
                den = epp.tile([128, 1], F32, tag="den")
                nc.scalar.activation(
                    E, as_ps[:, 0:T], mybir.ActivationFunctionType.Exp,
                    accum_out=den,
                )
                prod = epp.tile([128, T], F32, tag="prod")
                num = epp.tile([128, 1], F32, tag="num")
                nc.vector.tensor_mul(prod, E, as_ps[:, T:2 * T])
                nc.vector.reduce_sum(num, prod, axis=mybir.AxisListType.X)
                rden = epp.tile([128, 1], F32, tag="rden")
                nc.vector.reciprocal(rden, den)
                lg = epp.tile([128, 1], F32, tag="lg")
                nc.vector.tensor_mul(lg, num, rden)
                nc.scalar.activation(
                    out_sb[:, g:g + 1], lg,
                    mybir.ActivationFunctionType.Sigmoid,
                    bias=clab_sb, scale=1.0,
                )

            nc.sync.dma_start(out=out, in_=out_sb)

    nc.finalize()
    return nc


_NC_CACHE = None


def kernel(data_input, mlp_w, mlp_b, CM, attn, cla_w, cla_b):
    global LAST_RESULTS, _NC_CACHE

    data_input = np.ascontiguousarray(np.asarray(data_input, dtype=np.float32))
    mlp_w = np.asarray(mlp_w, dtype=np.float32)
    mlp_b = np.ascontiguousarray(np.asarray(mlp_b, dtype=np.float32))
    CM = np.asarray(CM, dtype=np.float32)
    attn_np = np.ascontiguousarray(np.asarray(attn, dtype=np.float32)).astype(NP_MM)
    cla_w = np.ascontiguousarray(np.asarray(cla_w, dtype=np.float32).reshape(H))
    cla_b = np.ascontiguousarray(np.asarray(cla_b, dtype=np.float32).reshape(1))

    wT = np.ascontiguousarray(mlp_w.T).astype(NP_MM)             # [D, H]
    in_maps = []
    if USE_BF16:
        cmn = CM.astype(NP_MM)
        cla_np = cla_w.astype(NP_MM)
        for i in range(NCORES):
            xT_i = np.ascontiguousarray(
                data_input[i * BLOC:(i + 1) * BLOC].T).astype(NP_MM)
            in_maps.append({
                "xT": xT_i, "wT": wT, "cmn": cmn, "attn": attn_np,
                "mlp_b": mlp_b, "cla_w": cla_np, "cla_b": cla_b,
            })
    else:
        cmt = np.ascontiguousarray(
            CM.transpose(2, 0, 1).reshape(H, T * H)
        ).astype(NP_MM)
        claM = np.zeros((128, KC, 2 * T), dtype=np.float32)
        claM[:, :, T] = cla_w.reshape(KC, 128).T
        claM = claM.astype(NP_MM)
        for i in range(NCORES):
            xT_i = np.ascontiguousarray(
                data_input[i * BLOC:(i + 1) * BLOC].T).astype(NP_MM)
            in_maps.append({
                "xT": xT_i, "wT": wT, "cmt": cmt, "attn": attn_np,
                "mlp_b": mlp_b, "claM": claM, "cla_b": cla_b,
            })

    if _NC_CACHE is None:
        _NC_CACHE = _build_nc_bf16() if USE_BF16 else _build_nc_f32r()

    trace = bool(int(os.environ.get("KERNEL_TRACE", "0")))
    res = run_bass_kernel_spmd(
        _NC_CACHE, in_maps, core_ids=list(range(NCORES)), trace=trace,
        trace_cores=[0] if trace else None,
    )
    LAST_RESULTS = res

    full = np.empty(B, dtype=np.float32)
    for i in range(NCORES):
        full[i * BLOC:(i + 1) * BLOC] = res.results[i]["out"].T.reshape(BLOC)
    return full


# revision 11
# speedup vs baseline: 1.7531x; 1.2437x over previous
# Trainium2 Bass kernel for nn_CN_MLP_71631464563230 (moe_routing).
#
# Math (after folding the classifier into the mixture):
#   mlp_out = x @ W.T + b                      [B, H]
#   a       = mlp_out @ attn                   [B, T]
#   V[t,h]  = sum_k CM[t,h,k] * cla_w[k]       [T, H]   (computed on device)
#   s       = mlp_out @ V.T                    [B, T]
#   out[b]  = sigmoid( (sum_t e^{a_bt} s_bt) / (sum_t e^{a_bt}) + cla_b )
#
# Sharding: data-parallel over B across 8 cores (1024 rows/core); params
# replicated. Host-side prep is layout-only (transposes + optional dtype
# narrowing) so contraction dims land on SBUF partitions.

import os

import ml_dtypes
import numpy as np

import concourse.bass as bass
import concourse.mybir as mybir
import concourse.tile as tile
from concourse import bacc
from concourse.bass_utils import run_bass_kernel_spmd
from concourse.masks import make_identity

B, D, H, T = 8192, 5000, 512, 16
NCORES = 8
BLOC = B // NCORES            # 1024 batch rows per core
NBSUB = BLOC // 128           # 8 b-subtiles per core
NHALF = 2                     # process b in two halves of 512
HALFW = BLOC // NHALF         # 512
KT = (D + 127) // 128         # 40 k-tiles over D
HT = H // 128                 # 4 h-tiles
KC = H // 128                 # 4 k-tiles over H (for V)
F32 = mybir.dt.float32

USE_BF16 = os.environ.get("KERNEL_BF16", "1") == "1"
MM_DT = mybir.dt.bfloat16 if USE_BF16 else mybir.dt.float32r
NP_MM = ml_dtypes.bfloat16 if USE_BF16 else np.float32

LAST_RESULTS = None           # BassKernelResults from the most recent run


def _build_nc_f32r():
    nc = bacc.Bacc("TRN2", target_bir_lowering=False)

    xT = nc.dram_tensor("xT", [D, BLOC], MM_DT, kind="ExternalInput").ap()
    wT = nc.dram_tensor("wT", [D, H], MM_DT, kind="ExternalInput").ap()
    cmt = nc.dram_tensor("cmt", [H, T * H], MM_DT, kind="ExternalInput").ap()
    attn = nc.dram_tensor("attn", [H, T], MM_DT, kind="ExternalInput").ap()
    mlpb = nc.dram_tensor("mlp_b", [H], F32, kind="ExternalInput").ap()
    claM_d = nc.dram_tensor("claM", [128, KC, 2 * T], MM_DT, kind="ExternalInput").ap()
    clab = nc.dram_tensor("cla_b", [1], F32, kind="ExternalInput").ap()
    out = nc.dram_tensor("out", [128, NBSUB], F32, kind="ExternalOutput").ap()

    with tile.TileContext(nc) as tc:
        import contextlib

        ctx = contextlib.ExitStack()
        with ctx:
            singles = ctx.enter_context(tc.tile_pool(name="singles", bufs=1))
            xtp = ctx.enter_context(tc.tile_pool(name="xt", bufs=8))
            wtp = ctx.enter_context(tc.tile_pool(name="wt", bufs=1))
            cmtp = ctx.enter_context(tc.tile_pool(name="cmt", bufs=3))
            mop = ctx.enter_context(tc.tile_pool(name="mo", bufs=2))
            epp = ctx.enter_context(tc.tile_pool(name="ep", bufs=4))
            mmp = ctx.enter_context(tc.tile_pool(name="mm", bufs=2, space="PSUM"))
            vpsp = ctx.enter_context(tc.tile_pool(name="vps", bufs=1, space="PSUM"))
            tpp = ctx.enter_context(tc.tile_pool(name="tp", bufs=1, space="PSUM"))
            asp = ctx.enter_context(tc.tile_pool(name="as", bufs=2, space="PSUM"))

            # ---- small constants -------------------------------------------
            # attn+VT combined rhs per h-tile: cols 0:16 = attn, 16:32 = V.T
            as_rhs = []
            for h in range(HT):
                t_ = singles.tile([128, 2 * T], MM_DT, tag=f"asrhs{h}", name=f"asrhs{h}")
                nc.sync.dma_start(out=t_[:, 0:T], in_=attn[h * 128:(h + 1) * 128, :])
                as_rhs.append(t_)

            biasT = singles.tile([128, HT], F32, tag="biasT")
            nc.sync.dma_start(out=biasT, in_=mlpb.rearrange("(a p) -> p a", p=128))

            # cla_w chunks embedded (host-side) in a zero buffer so an offset
            # slice gives a [128, T] lhsT with cla in column t, zeros elsewhere.
            claM = singles.tile([128, KC, 2 * T], MM_DT, tag="claM")
            nc.sync.dma_start(out=claM, in_=claM_d)

            clab_sb = singles.tile([128, 1], F32, tag="clab")
            nc.gpsimd.dma_start(
                out=clab_sb,
                in_=bass.AP(tensor=clab.tensor, offset=0, ap=[[0, 128], [1, 1]]),
            )

            ident = singles.tile([T, T], MM_DT, tag="ident")
            make_identity(nc, ident)

            v_sb = singles.tile([T, H], MM_DT, tag="v_sb")
            out_sb = singles.tile([128, NBSUB], F32, tag="out_sb")

            v_ps = vpsp.tile([T, H], F32, tag="v_ps")
            wt_tiles = [None] * KT
            mo = {}
            mm_ps = {}

            # V work interleaved into the main loop: one t-group every 5th
            # (half, k) slot keeps cmt DMA spread out and the PE warm.
            def v_group(t):
                cmt_t = cmtp.tile([128, KC, H], MM_DT, tag="cmt", name=f"cmt{t}")
                nc.sync.dma_start(
                    out=cmt_t,
                    in_=cmt[:, t * H:(t + 1) * H].rearrange("(j p) h -> p j h", p=128),
                )
                for j in range(KC):
                    nc.tensor.matmul(
                        v_ps,
                        lhsT=claM[:, j, T - t:2 * T - t],
                        rhs=cmt_t[:, j, :],
                        start=(t == 0 and j == 0),
                        stop=(t == T - 1 and j == KC - 1),
                    )

            # ---- main matmul: mlp_outT[h, b] = sum_d wT[d,h] * xT[d,b] ------
            for half in range(NHALF):
                for h in range(HT):
                    mm_ps[(half, h)] = mmp.tile(
                        [128, HALFW], F32, tag=f"mm{h}", name=f"mmps{half}_{h}"
                    )
                for k in range(KT):
                    slot = half * KT + k
                    if slot % 5 == 0 and slot // 5 < T:
                        v_group(slot // 5)
                    dk = min(128, D - k * 128)
                    if half == 0:
                        wt_tiles[k] = wtp.tile([128, H], MM_DT, tag=f"wt{k}", name=f"wt{k}")
                        nc.sync.dma_start(
                            out=wt_tiles[k][:dk, :], in_=wT[k * 128:k * 128 + dk, :]
                        )
                    xt_t = xtp.tile([128, HALFW], MM_DT, tag="xt")
                    nc.sync.dma_start(
                        out=xt_t[:dk, :],
                        in_=xT[k * 128:k * 128 + dk,
                              half * HALFW:(half + 1) * HALFW],
                    )
                    for h in range(HT):
                        nc.tensor.matmul(
                            mm_ps[(half, h)],
                            lhsT=wt_tiles[k][:dk, h * 128:(h + 1) * 128],
                            rhs=xt_t[:dk, :],
                            start=(k == 0),
                            stop=(k == KT - 1),
                        )
                for h in range(HT):
                    m = mop.tile([128, HALFW], MM_DT, tag=f"mo{h}", name=f"mo{half}_{h}")
                    nc.vector.tensor_scalar_add(m, mm_ps[(half, h)], biasT[:, h:h + 1])
                    mo[(half, h)] = m

            # ---- finish V: copy to SBUF, transpose into as_rhs cols 16:32 ---
            nc.vector.tensor_copy(v_sb, v_ps)
            for h in range(HT):
                tp_ps = tpp.tile([128, T], MM_DT, tag="tp")
                nc.tensor.transpose(tp_ps, v_sb[:, h * 128:(h + 1) * 128], ident)
                nc.vector.tensor_copy(as_rhs[h][:, T:2 * T], tp_ps)

            # ---- epilogue: a | s, softmax-combine, sigmoid ------------------
            for half in range(NHALF):
                for jl in range(NBSUB // NHALF):
                    g = half * (NBSUB // NHALF) + jl
                    as_ps = asp.tile([128, 2 * T], F32, tag="as")
                    for h in range(HT):
                        nc.tensor.matmul(
                            as_ps,
                            lhsT=mo[(half, h)][:, jl * 128:(jl + 1) * 128],
                            rhs=as_rhs[h],
                            start=(h == 0),
                            stop=(h == HT - 1),
                        )
                    E = epp.tile([128, T], F32, tag="E")
                    den = epp.tile([128, 1], F32, tag="den")
                    nc.scalar.activation(
                        E, as_ps[:, 0:T], mybir.ActivationFunctionType.Exp,
                        accum_out=den,
                    )
                    prod = epp.tile([128, T], F32, tag="prod")
                    num = epp.tile([128, 1], F32, tag="num")
                    nc.vector.tensor_mul(prod, E, as_ps[:, T:2 * T])
                    nc.vector.reduce_sum(num, prod, axis=mybir.AxisListType.X)
                    rden = epp.tile([128, 1], F32, tag="rden")
                    nc.vector.reciprocal(rden, den)
                    lg = epp.tile([128, 1], F32, tag="lg")
                    nc.vector.tensor_mul(lg, num, rden)
                    nc.scalar.activation(
                        out_sb[:, g:g + 1], lg,
                        mybir.ActivationFunctionType.Sigmoid,
                        bias=clab_sb, scale=1.0,
                    )

            nc.sync.dma_start(out=out, in_=out_sb)

    nc.finalize()
    return nc


def _build_nc_bf16():
    BF = mybir.dt.bfloat16
    nc = bacc.Bacc("TRN2", target_bir_lowering=False)

    # host-packed, partition-major layouts (see kernel() below)
    xT = nc.dram_tensor("xT", [128, KT * BLOC], BF, kind="ExternalInput").ap()
    wT = nc.dram_tensor("wT", [128, KT * H], BF, kind="ExternalInput").ap()
    cmn = nc.dram_tensor("cmn", [128, T * H * HT // 128 * 128], BF, kind="ExternalInput").ap()
    attn = nc.dram_tensor("attn", [H, T], BF, kind="ExternalInput").ap()
    mlpb = nc.dram_tensor("mlp_b", [H], F32, kind="ExternalInput").ap()
    claw = nc.dram_tensor("cla_w", [H], BF, kind="ExternalInput").ap()
    clab = nc.dram_tensor("cla_b", [1], F32, kind="ExternalInput").ap()
    out = nc.dram_tensor("out", [128, NBSUB], F32, kind="ExternalOutput").ap()

    XCHUNK = 2          # k-tiles per xt DMA
    WGRP = 5            # k-tiles per wt DMA
    CMCHUNK = 2         # t's per cm DMA

    with tile.TileContext(nc) as tc:
        import contextlib

        ctx = contextlib.ExitStack()
        with ctx:
            singles = ctx.enter_context(tc.tile_pool(name="singles", bufs=1))
            xtp = ctx.enter_context(tc.tile_pool(name="xt", bufs=6))
            wtp = ctx.enter_context(tc.tile_pool(name="wt", bufs=1))
            cmp_ = ctx.enter_context(tc.tile_pool(name="cm", bufs=2))
            vprodp = ctx.enter_context(tc.tile_pool(name="vprod", bufs=3))
            mop = ctx.enter_context(tc.tile_pool(name="mo", bufs=1))
            epp = ctx.enter_context(tc.tile_pool(name="ep", bufs=4))
            mmp = ctx.enter_context(tc.tile_pool(name="mm", bufs=1, space="PSUM"))

            # first compute tiles' DMAs lead the queue for a fast start
            wt_g = [None] * ((KT + WGRP - 1) // WGRP)
            wt_g[0] = wtp.tile([128, WGRP * H], BF, tag="wtg0", name="wtg0")
            nc.sync.dma_start(out=wt_g[0], in_=wT[:, 0:WGRP * H])
            xt_c = [None] * (KT // XCHUNK)
            xt_c[0] = xtp.tile([128, XCHUNK * BLOC], BF, tag="xt", name="xtc0")
            nc.sync.dma_start(out=xt_c[0], in_=xT[:, 0:XCHUNK * BLOC])

            # ---- small constants -------------------------------------------
            as_rhs = []
            for h in range(HT):
                t_ = singles.tile([128, 2 * T], BF, tag=f"asrhs{h}", name=f"asrhs{h}")
                nc.sync.dma_start(out=t_[:, 0:T], in_=attn[h * 128:(h + 1) * 128, :])
                as_rhs.append(t_)

            biasT = singles.tile([128, HT], F32, tag="biasT")
            nc.sync.dma_start(out=biasT, in_=mlpb.rearrange("(a p) -> p a", p=128))

            clab_sb = singles.tile([128, 1], F32, tag="clab")
            nc.gpsimd.dma_start(
                out=clab_sb,
                in_=bass.AP(tensor=clab.tensor, offset=0, ap=[[0, 128], [1, 1]]),
            )

            cla_rep = singles.tile([128, H], BF, tag="cla_rep")
            nc.sync.dma_start(out=cla_rep[0:1, :], in_=claw[None, :])
            nc.gpsimd.partition_broadcast(cla_rep, cla_rep[0:1, :])

            vt_f32 = []
            for h in range(HT):
                v_ = singles.tile([128, T], F32, tag=f"vt{h}", name=f"vt{h}")
                vt_f32.append(v_)

            out_sb = singles.tile([128, NBSUB], F32, tag="out_sb")

            # V on DVE: VT[h, t] = sum_k CM[t, h, k] * cla_w[k]
            cm_tiles = {}

            def v_chunk_dma(c):
                t_ = cmp_.tile([128, CMCHUNK * HT * H], BF, tag="cm", name=f"cm{c}")
                nc.sync.dma_start(
                    out=t_, in_=cmn[:, c * CMCHUNK * HT * H:(c + 1) * CMCHUNK * HT * H]
                )
                cm_tiles[c] = t_

            def v_unit(c):
                cm_t = cm_tiles[c]
                for tl in range(CMCHUNK):
                    t = c * CMCHUNK + tl
                    for j in range(HT):
                        prod = vprodp.tile([128, H], BF, tag="vprod", name=f"vp{t}_{j}")
                        nc.vector.tensor_mul(
                            prod, cm_t[:, (tl * HT + j) * H:(tl * HT + j + 1) * H],
                            cla_rep,
                        )
                        nc.vector.reduce_sum(
                            vt_f32[j][:, t:t + 1], prod, axis=mybir.AxisListType.X
                        )

            # ---- main matmul, single pass over k, all 8 psum banks ---------
            mm_ps = {}
            for i in range(2 * HT):
                mm_ps[i] = mmp.tile([128, HALFW], F32, tag=f"pm{i}", name=f"pmps{i}")
            for k in range(KT):
                c = k // XCHUNK
                lk = k % XCHUNK
                if lk == 0 and c + 1 < KT // XCHUNK:
                    xt_c[c + 1] = xtp.tile([128, XCHUNK * BLOC], BF, tag="xt",
                                           name=f"xtc{c + 1}")
                    nc.sync.dma_start(
                        out=xt_c[c + 1],
                        in_=xT[:, (c + 1) * XCHUNK * BLOC:(c + 2) * XCHUNK * BLOC],
                    )
                g = k // WGRP
                if k % WGRP == 0 and g + 1 < len(wt_g):
                    wt_g[g + 1] = wtp.tile([128, WGRP * H], BF, tag=f"wtg{g + 1}",
                                           name=f"wtg{g + 1}")
                    nc.sync.dma_start(
                        out=wt_g[g + 1],
                        in_=wT[:, (g + 1) * WGRP * H:(g + 2) * WGRP * H],
                    )
                if k % 4 == 0 and k // 4 < T // CMCHUNK:
                    v_chunk_dma(k // 4)
                if k % 4 == 2 and k // 4 < T // CMCHUNK:
                    v_unit(k // 4)
                if k == 34:
                    for h in range(HT):
                        nc.vector.tensor_copy(as_rhs[h][:, T:2 * T], vt_f32[h])
                dk = min(128, D - k * 128)
                for h in range(HT):
                    wsl = wt_g[g][:dk, (k % WGRP) * H + h * 128:
                                   (k % WGRP) * H + (h + 1) * 128]
                    for half in range(2):
                        nc.tensor.matmul(
                            mm_ps[h * 2 + half],
                            lhsT=wsl,
                            rhs=xt_c[c][:dk, lk * BLOC + half * HALFW:
                                        lk * BLOC + (half + 1) * HALFW],
                            start=(k == 0),
                            stop=(k == KT - 1),
                        )

            mo = {}
            for h in range(HT):
                m = mop.tile([128, BLOC], BF, tag=f"mo{h}", name=f"mo{h}")
                for half in range(2):
                    nc.vector.tensor_scalar_add(
                        m[:, half * HALFW:(half + 1) * HALFW],
                        mm_ps[h * 2 + half], biasT[:, h:h + 1],
                    )
                mo[h] = m

            # ---- epilogue: a | s, softmax-combine, sigmoid ------------------
            for g in range(NBSUB):
                as_ps = mmp.tile([128, 2 * T], F32, tag=f"pm{g}", name=f"as{g}")
                for h in range(HT):
                    nc.tensor.matmul(
                        as_ps,
                        lhsT=mo[h][:, g * 128:(g + 1) * 128],
                        rhs=as_rhs[h],
                        start=(h == 0),
                        stop=(h == HT - 1),
                    )
                E = epp.tile([128, T], F32, tag="E")
                den = epp.tile([128, 1], F32, tag="den")
                nc.scalar.activation(
                    E, as_ps[:, 0:T], mybir.ActivationFunctionType.Exp,
                    accum_out=den,
                )
                prod = epp.tile([128, T], F32, tag="prod")
                num = epp.tile([128, 1], F32, tag="num")
                nc.vector.tensor_mul(prod, E, as_ps[:, T:2 * T])
                nc.vector.reduce_sum(num, prod, axis=mybir.AxisListType.X)
                rden = epp.tile([128, 1], F32, tag="rden")
                nc.vector.reciprocal(rden, den)
                lg = epp.tile([128, 1], F32, tag="lg")
                nc.vector.tensor_mul(lg, num, rden)
                nc.scalar.activation(
                    out_sb[:, g:g + 1], lg,
                    mybir.ActivationFunctionType.Sigmoid,
                    bias=clab_sb, scale=1.0,
                )

            nc.sync.dma_start(out=out, in_=out_sb)

    nc.finalize()
    return nc


_NC_CACHE = None


def kernel(data_input, mlp_w, mlp_b, CM, attn, cla_w, cla_b):
    global LAST_RESULTS, _NC_CACHE

    data_input = np.ascontiguousarray(np.asarray(data_input, dtype=np.float32))
    mlp_w = np.asarray(mlp_w, dtype=np.float32)
    mlp_b = np.ascontiguousarray(np.asarray(mlp_b, dtype=np.float32))
    CM = np.asarray(CM, dtype=np.float32)
    attn_np = np.ascontiguousarray(np.asarray(attn, dtype=np.float32)).astype(NP_MM)
    cla_w = np.ascontiguousarray(np.asarray(cla_w, dtype=np.float32).reshape(H))
    cla_b = np.ascontiguousarray(np.asarray(cla_b, dtype=np.float32).reshape(1))

    wT = np.ascontiguousarray(mlp_w.T)                           # [D, H] f32
    in_maps = []
    if USE_BF16:
        wt_packed = np.zeros((128, KT * H), dtype=np.float32)
        for k in range(KT):
            dk = min(128, D - k * 128)
            wt_packed[:dk, k * H:(k + 1) * H] = wT[k * 128:k * 128 + dk, :]
        wt_packed = wt_packed.astype(NP_MM)
        cm_packed = np.empty((128, T * HT * H), dtype=np.float32)
        for t in range(T):
            for j in range(HT):
                cm_packed[:, (t * HT + j) * H:(t * HT + j + 1) * H] = \
                    CM[t, j * 128:(j + 1) * 128, :]
        cm_packed = cm_packed.astype(NP_MM)
        cla_np = cla_w.astype(NP_MM)
        for i in range(NCORES):
            xs = data_input[i * BLOC:(i + 1) * BLOC]
            xt_packed = np.zeros((128, KT * BLOC), dtype=np.float32)
            for k in range(KT):
                dk = min(128, D - k * 128)
                xt_packed[:dk, k * BLOC:(k + 1) * BLOC] = xs[:, k * 128:k * 128 + dk].T
            xt_packed = xt_packed.astype(NP_MM)
            in_maps.append({
                "xT": xt_packed, "wT": wt_packed, "cmn": cm_packed, "attn": attn_np,
                "mlp_b": mlp_b, "cla_w": cla_np, "cla_b": cla_b,
            })
    else:
        cmt = np.ascontiguousarray(
            CM.transpose(2, 0, 1).reshape(H, T * H)
        ).astype(NP_MM)
        claM = np.zeros((128, KC, 2 * T), dtype=np.float32)
        claM[:, :, T] = cla_w.reshape(KC, 128).T
        claM = claM.astype(NP_MM)
        wT_c = wT.astype(NP_MM)
        for i in range(NCORES):
            xT_i = np.ascontiguousarray(
                data_input[i * BLOC:(i + 1) * BLOC].T).astype(NP_MM)
            in_maps.append({
                "xT": xT_i, "wT": wT_c, "cmt": cmt, "attn": attn_np,
                "mlp_b": mlp_b, "claM": claM, "cla_b": cla_b,
            })

    if _NC_CACHE is None:
        _NC_CACHE = _build_nc_bf16() if USE_BF16 else _build_nc_f32r()

    trace = bool(int(os.environ.get("KERNEL_TRACE", "0")))
    res = run_bass_kernel_spmd(
        _NC_CACHE, in_maps, core_ids=list(range(NCORES)), trace=trace,
        trace_cores=[0] if trace else None,
    )
    LAST_RESULTS = res

    full = np.empty(B, dtype=np.float32)
    for i in range(NCORES):
        full[i * BLOC:(i + 1) * BLOC] = res.results[i]["out"].T.reshape(BLOC)
    return full
